# revision 4
# baseline (speedup 1.0000x reference)
"""Trainium2 Bass kernel for nn_GATrAutoRegressorLoss.

Strategy (data-parallel over the hit axis N, 8 cores):
  - The dominant cost is the assignment BCE over (T=32, N=500000) logits.
    Each core gets H = N/8 = 62500 hits, packed as a (128, 15625) f32 tile
    layout: partition p = j*32 + t, column f, hit = j*15625 + f.
  - Masks are folded into the logits via PE matmuls with host-built fp8
    one-hot matrices, then two ACT passes compute softplus = ln(1+exp(.))
    with a free running row-sum (accum_out):
      psumA = x + L^T @ E   where E one-hot encodes c(hit) = #valid steps
                            and L[k, t] = -96 * (t >= k)  (block-diag over j)
        -> exp underflows to 0 for masked elements, ln(1+0) = 0 exactly.
      psumB = x + 192 * D   where D one-hot selects the target row t = p(hit)
                            (only where the target is valid)
        -> relu(psumB - 96) = x + 96 at selected elements, 0 elsewhere; a
           single scalar_tensor_tensor gives the row-sums; the host subtracts
           96 * n_selected to recover sum_sel x (the BCE "- x*z" term).
  - The small (T,B) losses (dir/mag/pid/charge/stop) are computed on-device
    from host-scattered dense planes; index bookkeeping (bincount, cumcount,
    scatter, argmax one-hots, denominators) is host-side numpy.
  - Per-core partial sums are returned and combined on the host in float64.
"""

import numpy as np

import concourse.bacc as bacc
import concourse.mybir as mybir
from concourse.tile import TileContext
from concourse.bass_utils import run_bass_kernel_spmd

F32 = mybir.dt.float32
F8 = mybir.dt.float8e4
NP_F8 = mybir.dt.np(F8)

T, B, N, NPFO = 32, 256, 500000, 4096
L_DIR, L_MAG, L_PID, L_CHG, L_ASN, L_STP = 1.0, 1.0, 1.0, 0.5, 1.0, 0.5

N_CORES = 8
H = N // N_CORES          # hits per core
J = 4                     # partition packing factor (J*T = 128)
HQ = H // J               # packed columns per core
P = J * T                 # 128 partitions
FCH = 1024                # chunk width (columns)
MMW = 512                 # fp32 matmul moving-operand limit
PEN = 96.0                # mask penalty; exp(x-96) underflows to 0
BON = 192.0               # selection bonus; relu(x+192-96) = x+96

_CHUNKS = []
_c0 = 0
while _c0 < HQ:
    _CHUNKS.append((_c0, min(FCH, HQ - _c0)))
    _c0 += FCH
NCH = len(_CHUNKS)
assert NCH <= 16

# small-loss planes, each (T*B,) flattened to (128, 64)
_PLANES = [
    "pm0", "pm1", "pm2", "gm0", "gm1", "gm2", "pp", "gp", "pch", "gch",
    "stopx", "stopz", "valid",
    "pid0", "pid1", "pid2", "pid3", "pid4",
    "poh0", "poh1", "poh2", "poh3", "poh4",
]
NPL = len(_PLANES)
SW = 64  # small-plane free width (T*B = 8192 = 128*64)

_nc_cache = None
last_result = None


def _gen():
    nc = bacc.Bacc(None, target_bir_lowering=False, debug=True)
    x = nc.dram_tensor("x", [P, HQ], F32, kind="ExternalInput")
    e8 = nc.dram_tensor("e8", [P, HQ], F8, kind="ExternalInput")
    d8 = nc.dram_tensor("d8", [P, HQ], F8, kind="ExternalInput")
    l8 = nc.dram_tensor("l8", [P, P], F8, kind="ExternalInput")
    i192 = nc.dram_tensor("i192", [P, P], F8, kind="ExternalInput")
    idf = nc.dram_tensor("idf", [P, P], F32, kind="ExternalInput")
    sm = nc.dram_tensor("sm", [P, NPL * SW], F32, kind="ExternalInput")
    partials = nc.dram_tensor("partials", [P, 40], F32, kind="ExternalOutput")

    AF = mybir.ActivationFunctionType
    OP = mybir.AluOpType

    with TileContext(nc) as tc:
        with (
            tc.tile_pool(name="cst", bufs=1) as cst,
            tc.tile_pool(name="io", bufs=3) as io,
            tc.tile_pool(name="wk", bufs=2) as wk,
            tc.tile_pool(name="ps", bufs=2, space="PSUM") as ps,
            tc.tile_pool(name="sml", bufs=1) as sml,
        ):
            lt = cst.tile([P, P], F8)
            it = cst.tile([P, P], F8)
            ft = cst.tile([P, P], F32)
            zt = cst.tile([P, FCH], F32)
            accA = cst.tile([P, 16], F32)
            accB = cst.tile([P, 16], F32)
            accS = cst.tile([P, 8], F32)
            nc.sync.dma_start(out=lt[:], in_=l8[:])
            nc.sync.dma_start(out=it[:], in_=i192[:])
            nc.sync.dma_start(out=ft[:], in_=idf[:])
            nc.vector.memset(zt[:], 0.0)
            nc.vector.memset(accA[:], 0.0)
            nc.vector.memset(accB[:], 0.0)
            nc.vector.memset(accS[:], 0.0)

            # ---------------- main loop: assignment loss ----------------
            for ci, (c0, w) in enumerate(_CHUNKS):
                xt = io.tile([P, FCH], F32, tag="xt")
                et = io.tile([P, FCH], F8, tag="et")
                dt = io.tile([P, FCH], F8, tag="dt")
                nc.sync.dma_start(out=xt[:, :w], in_=x[:, c0 : c0 + w])
                nc.sync.dma_start(out=et[:, :w], in_=e8[:, c0 : c0 + w])
                nc.sync.dma_start(out=dt[:, :w], in_=d8[:, c0 : c0 + w])

                psA = ps.tile([P, FCH], F32, tag="psA")
                psB = ps.tile([P, FCH], F32, tag="psB")
                h0 = 0
                while h0 < w:
                    hw = min(MMW, w - h0)
                    sl = slice(h0, h0 + hw)
                    nc.tensor.matmul(
                        psA[:, sl], lt[:], et[:, sl], start=True, stop=False
                    )
                    nc.tensor.matmul(
                        psA[:, sl], ft[:], xt[:, sl], start=False, stop=True
                    )
                    nc.tensor.matmul(
                        psB[:, sl], it[:], dt[:, sl], start=True, stop=False
                    )
                    nc.tensor.matmul(
                        psB[:, sl], ft[:], xt[:, sl], start=False, stop=True
                    )
                    h0 += hw

                ut = wk.tile([P, FCH], F32, tag="ut")
                st = wk.tile([P, FCH], F32, tag="st")
                rt = wk.tile([P, FCH], F32, tag="rt")
                nc.scalar.activation(out=ut[:, :w], in_=psA[:, :w], func=AF.Exp)
                nc.scalar.activation(
                    out=st[:, :w],
                    in_=ut[:, :w],
                    func=AF.Ln,
                    bias=1.0,
                    accum_out=accA[:, ci : ci + 1],
                )
                nc.vector.scalar_tensor_tensor(
                    out=rt[:, :w],
                    in0=psB[:, :w],
                    scalar=-PEN,
                    in1=zt[:, :w],
                    op0=OP.add,
                    op1=OP.max,
                    accum_out=accB[:, ci : ci + 1],
                )

            # ---------------- small (T,B) losses ----------------
            smt = sml.tile([P, NPL * SW], F32)
            nc.sync.dma_start(out=smt[:], in_=sm[:])
            pl = {n: smt[:, i * SW : (i + 1) * SW] for i, n in enumerate(_PLANES)}

            _tmp_n = [0]

            def tmp():
                _tmp_n[0] += 1
                return sml.tile([P, SW], F32, name=f"tmp{_tmp_n[0]}", tag=f"tmp{_tmp_n[0]}")

            def sq_norm_inv(c0_, c1_, c2_):
                # 1 / max(sqrt(c0^2+c1^2+c2^2), 1e-8), sqrt via exp(0.5*ln)
                ss = tmp()
                t1 = tmp()
                nc.vector.tensor_mul(ss[:], c0_, c0_)
                nc.vector.tensor_mul(t1[:], c1_, c1_)
                nc.vector.tensor_add(ss[:], ss[:], t1[:])
                nc.vector.tensor_mul(t1[:], c2_, c2_)
                nc.vector.tensor_add(ss[:], ss[:], t1[:])
                lnv = tmp()
                nc.scalar.activation(out=lnv[:], in_=ss[:], func=AF.Ln)
                sr = tmp()
                nc.scalar.activation(out=sr[:], in_=lnv[:], func=AF.Exp, scale=0.5)
                nc.vector.tensor_scalar(
                    out=sr[:], in0=sr[:], scalar1=1e-8, scalar2=None, op0=OP.max
                )
                inv = tmp()
                nc.vector.reciprocal(out=inv[:], in_=sr[:])
                return inv

            # direction: sum(valid * (1 - cos))
            invp = sq_norm_inv(pl["pm0"], pl["pm1"], pl["pm2"])
            invg = sq_norm_inv(pl["gm0"], pl["gm1"], pl["gm2"])
            dot = tmp()
            t2 = tmp()
            nc.vector.tensor_mul(dot[:], pl["pm0"], pl["gm0"])
            nc.vector.tensor_mul(t2[:], pl["pm1"], pl["gm1"])
            nc.vector.tensor_add(dot[:], dot[:], t2[:])
            nc.vector.tensor_mul(t2[:], pl["pm2"], pl["gm2"])
            nc.vector.tensor_add(dot[:], dot[:], t2[:])
            nc.vector.tensor_mul(dot[:], dot[:], invp[:])
            nc.vector.tensor_mul(dot[:], dot[:], invg[:])  # cos
            cv = tmp()
            nc.vector.tensor_mul(cv[:], dot[:], pl["valid"])
            dsc = tmp()
            nc.vector.scalar_tensor_tensor(
                out=dsc[:], in0=cv[:], scalar=-1.0, in1=pl["valid"],
                op0=OP.mult, op1=OP.add, accum_out=accS[:, 0:1],
            )

            def masked_sq(a, b, col):
                d = tmp()
                nc.vector.tensor_sub(d[:], a, b)
                nc.vector.tensor_mul(d[:], d[:], d[:])
                o = tmp()
                nc.vector.scalar_tensor_tensor(
                    out=o[:], in0=d[:], scalar=1.0, in1=pl["valid"],
                    op0=OP.mult, op1=OP.mult, accum_out=accS[:, col : col + 1],
                )

            masked_sq(pl["pp"], pl["gp"], 1)       # magnitude
            masked_sq(pl["pch"], pl["gch"], 2)     # charge

            # pid: sum(valid * (lse + m - x_cls)), lse = ln(sum exp(x_k - m))
            pid = [pl[f"pid{k}"] for k in range(5)]
            poh = [pl[f"poh{k}"] for k in range(5)]
            m = tmp()
            nc.vector.tensor_max(m[:], pid[0], pid[1])
            for k in range(2, 5):
                nc.vector.tensor_max(m[:], m[:], pid[k])
            se = tmp()
            ek = tmp()
            sk = tmp()
            for k in range(5):
                nc.vector.tensor_sub(sk[:], pid[k], m[:])
                nc.scalar.activation(out=ek[:], in_=sk[:], func=AF.Exp)
                if k == 0:
                    nc.vector.tensor_copy(se[:], ek[:])
                else:
                    nc.vector.tensor_add(se[:], se[:], ek[:])
            lse = tmp()
            nc.scalar.activation(out=lse[:], in_=se[:], func=AF.Ln)
            xcls = tmp()
            tk = tmp()
            for k in range(5):
                nc.vector.tensor_mul(tk[:], pid[k], poh[k])
                if k == 0:
                    nc.vector.tensor_copy(xcls[:], tk[:])
                else:
                    nc.vector.tensor_add(xcls[:], xcls[:], tk[:])
            n1 = tmp()
            nc.vector.tensor_add(n1[:], lse[:], m[:])
            u = tmp()
            nc.vector.scalar_tensor_tensor(
                out=u[:], in0=xcls[:], scalar=-1.0, in1=n1[:],
                op0=OP.mult, op1=OP.add,
            )
            o = tmp()
            nc.vector.scalar_tensor_tensor(
                out=o[:], in0=u[:], scalar=1.0, in1=pl["valid"],
                op0=OP.mult, op1=OP.mult, accum_out=accS[:, 3:4],
            )

            # stop: sum over all of softplus(x) - x*z
            usp = tmp()
            spv = tmp()
            nc.scalar.activation(out=usp[:], in_=pl["stopx"], func=AF.Exp)
            nc.scalar.activation(out=spv[:], in_=usp[:], func=AF.Ln, bias=1.0)
            xz = tmp()
            nc.vector.tensor_mul(xz[:], pl["stopx"], pl["stopz"])
            o2 = tmp()
            nc.vector.scalar_tensor_tensor(
                out=o2[:], in0=xz[:], scalar=-1.0, in1=spv[:],
                op0=OP.mult, op1=OP.add, accum_out=accS[:, 4:5],
            )

            nc.sync.dma_start(out=partials[:, 0:16], in_=accA[:])
            nc.sync.dma_start(out=partials[:, 16:32], in_=accB[:])
            nc.sync.dma_start(out=partials[:, 32:40], in_=accS[:])
    nc.finalize()
    return nc


def _get_nc():
    global _nc_cache
    if _nc_cache is None:
        _nc_cache = _gen()
    return _nc_cache


def _cumcount(gb):
    n = gb.shape[0]
    order = np.argsort(gb, kind="stable")
    sb = gb[order]
    first = np.searchsorted(sb, sb, side="left")
    cum = np.arange(n) - first
    out = np.zeros(n, dtype=np.int64)
    out[order] = cum
    return out


def kernel(**inputs):
    pfo_momentum = np.asarray(inputs["pfo_momentum"], np.float32)
    pfo_p_mod = np.asarray(inputs["pfo_p_mod"], np.float32)
    pfo_pid = np.asarray(inputs["pfo_pid"], np.float32)
    pfo_charge = np.asarray(inputs["pfo_charge"], np.float32)
    al = np.asarray(inputs["assignments_logits"], np.float32).reshape(T, N)
    stop_logits = np.asarray(inputs["stop_logits"], np.float32)
    gt_momentum = np.asarray(inputs["gt_momentum"], np.float32)
    gt_p_mod = np.asarray(inputs["gt_p_mod"], np.float32)
    gt_pid = np.asarray(inputs["gt_pid"], np.float32)
    gt_charge = np.asarray(inputs["gt_charge"], np.float32)
    gt_batch = np.asarray(inputs["gt_batch"]).astype(np.int64)
    hit_to_pfo = np.asarray(inputs["hit_to_pfo"]).astype(np.int64)
    hit_batch = np.asarray(inputs["hit_batch"]).astype(np.int64)

    # ---- host index bookkeeping ----
    ppe = np.bincount(gt_batch, minlength=B)[:B]                  # (B,)
    cmin = np.minimum(ppe[hit_batch], T)                          # (N,)
    w = hit_to_pfo < cmin                                         # (N,) bool
    n_sel = int(w.sum())
    assign_den = max(float(cmin.sum()), 1.0)

    step_idx = _cumcount(gt_batch)
    keep = step_idx < T
    si, gb = step_idx[keep], gt_batch[keep]

    def scat(vals):
        out = np.zeros((T, B) + vals.shape[1:], np.float32)
        out[si, gb] = vals[keep]
        return out

    gt_mom_tb = scat(gt_momentum)
    gt_pmod_tb = scat(gt_p_mod)
    gt_pid_tb = scat(gt_pid)
    gt_chg_tb = scat(gt_charge)

    steps = np.arange(T)[:, None]
    valid = (steps < ppe[None, :]).astype(np.float32)             # (T,B)
    vcnt = max(float(valid.sum()), 1.0)
    gt_stop = (steps >= ppe[None, :]).astype(np.float32)
    gt_cls = np.argmax(gt_pid_tb, axis=-1)                        # (T,B)
    poh = np.zeros((T, B, 5), np.float32)
    np.put_along_axis(poh, gt_cls[..., None], 1.0, axis=-1)

    # ---- per-core device inputs ----
    def pack_plane(a):
        return np.ascontiguousarray(a.reshape(P, SW))

    planes = {
        "pm0": pfo_momentum[..., 0], "pm1": pfo_momentum[..., 1],
        "pm2": pfo_momentum[..., 2],
        "gm0": gt_mom_tb[..., 0], "gm1": gt_mom_tb[..., 1],
        "gm2": gt_mom_tb[..., 2],
        "pp": pfo_p_mod[..., 0], "gp": gt_pmod_tb[..., 0],
        "pch": pfo_charge[..., 0], "gch": gt_chg_tb[..., 0],
        "stopx": stop_logits[..., 0], "stopz": gt_stop,
        "valid": valid,
        **{f"pid{k}": pfo_pid[..., k] for k in range(5)},
        **{f"poh{k}": poh[..., k] for k in range(5)},
    }
    sm = np.concatenate([pack_plane(planes[n]) for n in _PLANES], axis=1)

    l8 = np.zeros((P, P), np.float32)
    for j in range(J):
        blk = -PEN * np.tril(np.ones((T, T), np.float32)).T  # [k,t] = -96*(t>=k)
        l8[j * T : (j + 1) * T, j * T : (j + 1) * T] = blk
    l8 = l8.astype(NP_F8)
    i192 = (BON * np.eye(P, dtype=np.float32)).astype(NP_F8)
    idf = np.eye(P, dtype=np.float32)

    # one-hot E (mask count) and D (selection) per core, fp8
    cj = cmin.reshape(N_CORES, J, HQ)
    pj = hit_to_pfo.reshape(N_CORES, J, HQ)
    wj = w.reshape(N_CORES, J, HQ)
    in_maps = []
    for c in range(N_CORES):
        E = np.zeros((P, HQ), NP_F8)
        D = np.zeros((P, HQ), NP_F8)
        for j in range(J):
            cc = cj[c, j]
            me = cc < T
            fs = np.nonzero(me)[0]
            E[j * T + cc[fs], fs] = 1.0
            fs = np.nonzero(wj[c, j])[0]
            D[j * T + pj[c, j][fs], fs] = 1.0
        xs = al[:, c * H : (c + 1) * H].reshape(T, J, HQ)
        xp = np.ascontiguousarray(xs.transpose(1, 0, 2).reshape(P, HQ))
        in_maps.append(
            {"x": xp, "e8": E, "d8": D, "l8": l8, "i192": i192, "idf": idf,
             "sm": sm}
        )

    nc = _get_nc()
    res = run_bass_kernel_spmd(nc, in_maps, core_ids=list(range(N_CORES)))
    global last_result
    last_result = res

    # ---- host combine (float64) ----
    A_sum = 0.0
    B_sum = 0.0
    for c in range(N_CORES):
        pr = res.results[c]["partials"].astype(np.float64)
        A_sum += pr[:, 0:16].sum()
        B_sum += pr[:, 16:32].sum()
    selx = B_sum - PEN * n_sel
    loss_assign = (A_sum - selx) / assign_den

    pr0 = res.results[0]["partials"].astype(np.float64)
    loss_dir = pr0[:, 32].sum() / vcnt
    loss_mag = pr0[:, 33].sum() / vcnt
    loss_chg = pr0[:, 34].sum() / vcnt
    loss_pid = pr0[:, 35].sum() / vcnt
    loss_stop = pr0[:, 36].sum() / (T * B)

    total = (L_DIR * loss_dir + L_MAG * loss_mag + L_PID * loss_pid
             + L_CHG * loss_chg + L_ASN * loss_assign + L_STP * loss_stop)
    f = np.float32
    return (f(total), f(loss_dir), f(loss_mag), f(loss_pid), f(loss_chg),
            f(loss_assign), f(loss_stop))


# revision 5
# speedup vs baseline: 1.2607x; 1.2607x over previous
"""Trainium2 Bass kernel for nn_GATrAutoRegressorLoss.

Strategy (data-parallel over the hit axis N, 8 cores):
  - The dominant cost is the assignment BCE over (T=32, N=500000) logits.
    Each core gets H = N/8 = 62500 hits, packed as a (128, 15625) f32 tile
    layout: partition p = j*32 + t, column f, hit = j*15625 + f.
  - Masks are folded into the logits via PE matmuls with host-built fp8
    one-hot matrices, then two ACT passes compute softplus = ln(1+exp(.))
    with a free running row-sum (accum_out):
      psumA = x + L^T @ E   where E one-hot encodes c(hit) = #valid steps
                            and L[k, t] = -96 * (t >= k)  (block-diag over j)
        -> exp underflows to 0 for masked elements, ln(1+0) = 0 exactly.
      sum_sel x (the BCE "- x*z" term) needs no second psum: selected
      elements are valid, so psumA = x there, and one scalar_tensor_tensor
      psumA * D (D the fp8 one-hot selector read straight from SBUF) with
      accum_out gives the row-sums exactly.
  - The small (T,B) losses (dir/mag/pid/charge/stop) are computed on-device
    from host-scattered dense planes; index bookkeeping (bincount, cumcount,
    scatter, argmax one-hots, denominators) is host-side numpy.
  - Per-core partial sums are returned and combined on the host in float64.
"""

import numpy as np

import concourse.bacc as bacc
import concourse.mybir as mybir
from concourse.tile import TileContext
from concourse.bass_utils import run_bass_kernel_spmd

F32 = mybir.dt.float32
F8 = mybir.dt.float8e4
NP_F8 = mybir.dt.np(F8)

T, B, N, NPFO = 32, 256, 500000, 4096
L_DIR, L_MAG, L_PID, L_CHG, L_ASN, L_STP = 1.0, 1.0, 1.0, 0.5, 1.0, 0.5

N_CORES = 8
H = N // N_CORES          # hits per core
J = 4                     # partition packing factor (J*T = 128)
HQ = H // J               # packed columns per core
P = J * T                 # 128 partitions
FCH = 2048                # chunk width (columns)
MMW = 512                 # fp32 matmul moving-operand limit
PEN = 96.0                # mask penalty; exp(x-96) underflows to 0
NGRP = 2                  # chunk groups; exp/ln batched per group (ACT tables)

_CHUNKS = []
_c0 = 0
while _c0 < HQ:
    _CHUNKS.append((_c0, min(FCH, HQ - _c0)))
    _c0 += FCH
NCH = len(_CHUNKS)
assert NCH <= 16

# small-loss planes, each (T*B,) flattened to (128, 64)
_PLANES = [
    "pm0", "pm1", "pm2", "gm0", "gm1", "gm2", "pp", "gp", "pch", "gch",
    "stopx", "stopz", "valid",
    "pid0", "pid1", "pid2", "pid3", "pid4",
    "poh0", "poh1", "poh2", "poh3", "poh4",
]
NPL = len(_PLANES)
SW = 64  # small-plane free width (T*B = 8192 = 128*64)

_nc_cache = None
last_result = None


def _gen():
    nc = bacc.Bacc(None, target_bir_lowering=False, debug=True)
    x = nc.dram_tensor("x", [P, HQ], F32, kind="ExternalInput")
    e8 = nc.dram_tensor("e8", [P, HQ], F8, kind="ExternalInput")
    d8 = nc.dram_tensor("d8", [P, HQ], F8, kind="ExternalInput")
    l8 = nc.dram_tensor("l8", [P, P], F8, kind="ExternalInput")
    idf = nc.dram_tensor("idf", [P, P], F32, kind="ExternalInput")
    sm = nc.dram_tensor("sm", [P, NPL * SW], F32, kind="ExternalInput")
    partials = nc.dram_tensor("partials", [P, 40], F32, kind="ExternalOutput")

    AF = mybir.ActivationFunctionType
    OP = mybir.AluOpType

    with TileContext(nc) as tc:
        with (
            tc.tile_pool(name="cst", bufs=1) as cst,
            tc.tile_pool(name="io", bufs=3) as io,
            tc.tile_pool(name="wk", bufs=2) as wk,
            tc.tile_pool(name="ps", bufs=2, space="PSUM") as ps,
            tc.tile_pool(name="sml", bufs=1) as sml,
        ):
            lt = cst.tile([P, P], F8)
            ft = cst.tile([P, P], F32)
            accA = cst.tile([P, 16], F32)
            accB = cst.tile([P, 16], F32)
            accS = cst.tile([P, 8], F32)
            ubuf = cst.tile([P, HQ], F32)
            nc.sync.dma_start(out=lt[:], in_=l8[:])
            nc.sync.dma_start(out=ft[:], in_=idf[:])
            nc.vector.memset(accA[:], 0.0)
            nc.vector.memset(accB[:], 0.0)
            nc.vector.memset(accS[:], 0.0)

            # ---------------- main loop: assignment loss ----------------
            # exp passes write u = exp(x - 96*notM) into ubuf; ln passes are
            # batched per chunk-group so the ACT engine switches function
            # tables only twice per group instead of twice per chunk.
            groups = [
                list(range(g * len(_CHUNKS) // NGRP,
                           (g + 1) * len(_CHUNKS) // NGRP))
                for g in range(NGRP)
            ]
            for grp in groups:
                for ci in grp:
                    c0, w = _CHUNKS[ci]
                    xt = io.tile([P, FCH], F32, tag="xt")
                    et = io.tile([P, FCH], F8, tag="et")
                    dt = io.tile([P, FCH], F8, tag="dt")
                    nc.sync.dma_start(out=xt[:, :w], in_=x[:, c0 : c0 + w])
                    nc.gpsimd.dma_start(out=et[:, :w], in_=e8[:, c0 : c0 + w])
                    nc.gpsimd.dma_start(out=dt[:, :w], in_=d8[:, c0 : c0 + w])

                    psA = ps.tile([P, FCH], F32, tag="psA")
                    h0 = 0
                    while h0 < w:
                        hw = min(MMW, w - h0)
                        sl = slice(h0, h0 + hw)
                        nc.tensor.matmul(
                            psA[:, sl], lt[:], et[:, sl], start=True, stop=False
                        )
                        nc.tensor.matmul(
                            psA[:, sl], ft[:], xt[:, sl], start=False, stop=True
                        )
                        h0 += hw

                    nc.scalar.activation(
                        out=ubuf[:, c0 : c0 + w], in_=psA[:, :w], func=AF.Exp
                    )
                    rt = wk.tile([P, FCH], F32, tag="rt")
                    nc.vector.scalar_tensor_tensor(
                        out=rt[:, :w],
                        in0=psA[:, :w],
                        scalar=1.0,
                        in1=dt[:, :w],
                        op0=OP.mult,
                        op1=OP.mult,
                        accum_out=accB[:, ci : ci + 1],
                    )
                for ci in grp:
                    c0, w = _CHUNKS[ci]
                    st = wk.tile([P, FCH], F32, tag="st")
                    nc.scalar.activation(
                        out=st[:, :w],
                        in_=ubuf[:, c0 : c0 + w],
                        func=AF.Ln,
                        bias=1.0,
                        accum_out=accA[:, ci : ci + 1],
                    )

            # ---------------- small (T,B) losses ----------------
            smt = sml.tile([P, NPL * SW], F32)
            nc.sync.dma_start(out=smt[:], in_=sm[:])
            pl = {n: smt[:, i * SW : (i + 1) * SW] for i, n in enumerate(_PLANES)}

            _tmp_n = [0]

            def tmp():
                _tmp_n[0] += 1
                return sml.tile([P, SW], F32, name=f"tmp{_tmp_n[0]}", tag=f"tmp{_tmp_n[0]}")

            def sq_norm_inv(c0_, c1_, c2_):
                # 1 / max(sqrt(c0^2+c1^2+c2^2), 1e-8), sqrt via exp(0.5*ln)
                ss = tmp()
                t1 = tmp()
                nc.vector.tensor_mul(ss[:], c0_, c0_)
                nc.vector.tensor_mul(t1[:], c1_, c1_)
                nc.vector.tensor_add(ss[:], ss[:], t1[:])
                nc.vector.tensor_mul(t1[:], c2_, c2_)
                nc.vector.tensor_add(ss[:], ss[:], t1[:])
                lnv = tmp()
                nc.scalar.activation(out=lnv[:], in_=ss[:], func=AF.Ln)
                sr = tmp()
                nc.scalar.activation(out=sr[:], in_=lnv[:], func=AF.Exp, scale=0.5)
                nc.vector.tensor_scalar(
                    out=sr[:], in0=sr[:], scalar1=1e-8, scalar2=None, op0=OP.max
                )
                inv = tmp()
                nc.vector.reciprocal(out=inv[:], in_=sr[:])
                return inv

            # direction: sum(valid * (1 - cos))
            invp = sq_norm_inv(pl["pm0"], pl["pm1"], pl["pm2"])
            invg = sq_norm_inv(pl["gm0"], pl["gm1"], pl["gm2"])
            dot = tmp()
            t2 = tmp()
            nc.vector.tensor_mul(dot[:], pl["pm0"], pl["gm0"])
            nc.vector.tensor_mul(t2[:], pl["pm1"], pl["gm1"])
            nc.vector.tensor_add(dot[:], dot[:], t2[:])
            nc.vector.tensor_mul(t2[:], pl["pm2"], pl["gm2"])
            nc.vector.tensor_add(dot[:], dot[:], t2[:])
            nc.vector.tensor_mul(dot[:], dot[:], invp[:])
            nc.vector.tensor_mul(dot[:], dot[:], invg[:])  # cos
            cv = tmp()
            nc.vector.tensor_mul(cv[:], dot[:], pl["valid"])
            dsc = tmp()
            nc.vector.scalar_tensor_tensor(
                out=dsc[:], in0=cv[:], scalar=-1.0, in1=pl["valid"],
                op0=OP.mult, op1=OP.add, accum_out=accS[:, 0:1],
            )

            def masked_sq(a, b, col):
                d = tmp()
                nc.vector.tensor_sub(d[:], a, b)
                nc.vector.tensor_mul(d[:], d[:], d[:])
                o = tmp()
                nc.vector.scalar_tensor_tensor(
                    out=o[:], in0=d[:], scalar=1.0, in1=pl["valid"],
                    op0=OP.mult, op1=OP.mult, accum_out=accS[:, col : col + 1],
                )

            masked_sq(pl["pp"], pl["gp"], 1)       # magnitude
            masked_sq(pl["pch"], pl["gch"], 2)     # charge

            # pid: sum(valid * (lse + m - x_cls)), lse = ln(sum exp(x_k - m))
            pid = [pl[f"pid{k}"] for k in range(5)]
            poh = [pl[f"poh{k}"] for k in range(5)]
            m = tmp()
            nc.vector.tensor_max(m[:], pid[0], pid[1])
            for k in range(2, 5):
                nc.vector.tensor_max(m[:], m[:], pid[k])
            se = tmp()
            ek = tmp()
            sk = tmp()
            for k in range(5):
                nc.vector.tensor_sub(sk[:], pid[k], m[:])
                nc.scalar.activation(out=ek[:], in_=sk[:], func=AF.Exp)
                if k == 0:
                    nc.vector.tensor_copy(se[:], ek[:])
                else:
                    nc.vector.tensor_add(se[:], se[:], ek[:])
            lse = tmp()
            nc.scalar.activation(out=lse[:], in_=se[:], func=AF.Ln)
            xcls = tmp()
            tk = tmp()
            for k in range(5):
                nc.vector.tensor_mul(tk[:], pid[k], poh[k])
                if k == 0:
                    nc.vector.tensor_copy(xcls[:], tk[:])
                else:
                    nc.vector.tensor_add(xcls[:], xcls[:], tk[:])
            n1 = tmp()
            nc.vector.tensor_add(n1[:], lse[:], m[:])
            u = tmp()
            nc.vector.scalar_tensor_tensor(
                out=u[:], in0=xcls[:], scalar=-1.0, in1=n1[:],
                op0=OP.mult, op1=OP.add,
            )
            o = tmp()
            nc.vector.scalar_tensor_tensor(
                out=o[:], in0=u[:], scalar=1.0, in1=pl["valid"],
                op0=OP.mult, op1=OP.mult, accum_out=accS[:, 3:4],
            )

            # stop: sum over all of softplus(x) - x*z
            usp = tmp()
            spv = tmp()
            nc.scalar.activation(out=usp[:], in_=pl["stopx"], func=AF.Exp)
            nc.scalar.activation(out=spv[:], in_=usp[:], func=AF.Ln, bias=1.0)
            xz = tmp()
            nc.vector.tensor_mul(xz[:], pl["stopx"], pl["stopz"])
            o2 = tmp()
            nc.vector.scalar_tensor_tensor(
                out=o2[:], in0=xz[:], scalar=-1.0, in1=spv[:],
                op0=OP.mult, op1=OP.add, accum_out=accS[:, 4:5],
            )

            nc.sync.dma_start(out=partials[:, 0:16], in_=accA[:])
            nc.sync.dma_start(out=partials[:, 16:32], in_=accB[:])
            nc.sync.dma_start(out=partials[:, 32:40], in_=accS[:])
    nc.finalize()
    return nc


def _get_nc():
    global _nc_cache
    if _nc_cache is None:
        _nc_cache = _gen()
    return _nc_cache


def _cumcount(gb):
    n = gb.shape[0]
    order = np.argsort(gb, kind="stable")
    sb = gb[order]
    first = np.searchsorted(sb, sb, side="left")
    cum = np.arange(n) - first
    out = np.zeros(n, dtype=np.int64)
    out[order] = cum
    return out


def kernel(**inputs):
    pfo_momentum = np.asarray(inputs["pfo_momentum"], np.float32)
    pfo_p_mod = np.asarray(inputs["pfo_p_mod"], np.float32)
    pfo_pid = np.asarray(inputs["pfo_pid"], np.float32)
    pfo_charge = np.asarray(inputs["pfo_charge"], np.float32)
    al = np.asarray(inputs["assignments_logits"], np.float32).reshape(T, N)
    stop_logits = np.asarray(inputs["stop_logits"], np.float32)
    gt_momentum = np.asarray(inputs["gt_momentum"], np.float32)
    gt_p_mod = np.asarray(inputs["gt_p_mod"], np.float32)
    gt_pid = np.asarray(inputs["gt_pid"], np.float32)
    gt_charge = np.asarray(inputs["gt_charge"], np.float32)
    gt_batch = np.asarray(inputs["gt_batch"]).astype(np.int64)
    hit_to_pfo = np.asarray(inputs["hit_to_pfo"]).astype(np.int64)
    hit_batch = np.asarray(inputs["hit_batch"]).astype(np.int64)

    # ---- host index bookkeeping ----
    ppe = np.bincount(gt_batch, minlength=B)[:B]                  # (B,)
    cmin = np.minimum(ppe[hit_batch], T)                          # (N,)
    w = hit_to_pfo < cmin                                         # (N,) bool
    n_sel = int(w.sum())
    assign_den = max(float(cmin.sum()), 1.0)

    step_idx = _cumcount(gt_batch)
    keep = step_idx < T
    si, gb = step_idx[keep], gt_batch[keep]

    def scat(vals):
        out = np.zeros((T, B) + vals.shape[1:], np.float32)
        out[si, gb] = vals[keep]
        return out

    gt_mom_tb = scat(gt_momentum)
    gt_pmod_tb = scat(gt_p_mod)
    gt_pid_tb = scat(gt_pid)
    gt_chg_tb = scat(gt_charge)

    steps = np.arange(T)[:, None]
    valid = (steps < ppe[None, :]).astype(np.float32)             # (T,B)
    vcnt = max(float(valid.sum()), 1.0)
    gt_stop = (steps >= ppe[None, :]).astype(np.float32)
    gt_cls = np.argmax(gt_pid_tb, axis=-1)                        # (T,B)
    poh = np.zeros((T, B, 5), np.float32)
    np.put_along_axis(poh, gt_cls[..., None], 1.0, axis=-1)

    # ---- per-core device inputs ----
    def pack_plane(a):
        return np.ascontiguousarray(a.reshape(P, SW))

    planes = {
        "pm0": pfo_momentum[..., 0], "pm1": pfo_momentum[..., 1],
        "pm2": pfo_momentum[..., 2],
        "gm0": gt_mom_tb[..., 0], "gm1": gt_mom_tb[..., 1],
        "gm2": gt_mom_tb[..., 2],
        "pp": pfo_p_mod[..., 0], "gp": gt_pmod_tb[..., 0],
        "pch": pfo_charge[..., 0], "gch": gt_chg_tb[..., 0],
        "stopx": stop_logits[..., 0], "stopz": gt_stop,
        "valid": valid,
        **{f"pid{k}": pfo_pid[..., k] for k in range(5)},
        **{f"poh{k}": poh[..., k] for k in range(5)},
    }
    sm = np.concatenate([pack_plane(planes[n]) for n in _PLANES], axis=1)

    l8 = np.zeros((P, P), np.float32)
    for j in range(J):
        blk = -PEN * np.tril(np.ones((T, T), np.float32)).T  # [k,t] = -96*(t>=k)
        l8[j * T : (j + 1) * T, j * T : (j + 1) * T] = blk
    l8 = l8.astype(NP_F8)
    idf = np.eye(P, dtype=np.float32)

    # one-hot E (mask count) and D (selection) per core, fp8
    cj = cmin.reshape(N_CORES, J, HQ)
    pj = hit_to_pfo.reshape(N_CORES, J, HQ)
    wj = w.reshape(N_CORES, J, HQ)
    in_maps = []
    for c in range(N_CORES):
        E = np.zeros((P, HQ), NP_F8)
        D = np.zeros((P, HQ), NP_F8)
        for j in range(J):
            cc = cj[c, j]
            me = cc < T
            fs = np.nonzero(me)[0]
            E[j * T + cc[fs], fs] = 1.0
            fs = np.nonzero(wj[c, j])[0]
            D[j * T + pj[c, j][fs], fs] = 1.0
        xs = al[:, c * H : (c + 1) * H].reshape(T, J, HQ)
        xp = np.ascontiguousarray(xs.transpose(1, 0, 2).reshape(P, HQ))
        in_maps.append(
            {"x": xp, "e8": E, "d8": D, "l8": l8, "idf": idf, "sm": sm}
        )

    nc = _get_nc()
    res = run_bass_kernel_spmd(nc, in_maps, core_ids=list(range(N_CORES)))
    global last_result
    last_result = res

    # ---- host combine (float64) ----
    A_sum = 0.0
    B_sum = 0.0
    for c in range(N_CORES):
        pr = res.results[c]["partials"].astype(np.float64)
        A_sum += pr[:, 0:16].sum()
        B_sum += pr[:, 16:32].sum()
    loss_assign = (A_sum - B_sum) / assign_den

    pr0 = res.results[0]["partials"].astype(np.float64)
    loss_dir = pr0[:, 32].sum() / vcnt
    loss_mag = pr0[:, 33].sum() / vcnt
    loss_chg = pr0[:, 34].sum() / vcnt
    loss_pid = pr0[:, 35].sum() / vcnt
    loss_stop = pr0[:, 36].sum() / (T * B)

    total = (L_DIR * loss_dir + L_MAG * loss_mag + L_PID * loss_pid
             + L_CHG * loss_chg + L_ASN * loss_assign + L_STP * loss_stop)
    f = np.float32
    return (f(total), f(loss_dir), f(loss_mag), f(loss_pid), f(loss_chg),
            f(loss_assign), f(loss_stop))


# revision 8
# speedup vs baseline: 1.4287x; 1.1332x over previous
"""Trainium2 Bass kernel for nn_GATrAutoRegressorLoss.

Strategy (data-parallel over the hit axis N, 8 cores):
  - The dominant cost is the assignment BCE over (T=32, N=500000) logits.
    Each core gets H = N/8 = 62500 hits, packed as a (128, 15625) layout:
    partition p = j*32 + t, column f, hit = j*15625 + f.
  - The validity mask is folded into the logits on the PE: host-built fp8
    one-hot columns E (encoding c(hit) = #valid steps) hit a constant
    block-triangular L with value -96, accumulating -96*(t >= c) into PSUM;
    x rides in via two bf16 identity matmuls (hi + lo split preserves f32
    precision).  psA = x - 96*notM.
  - softplus = ln(1 + exp(.)) as two ACT passes (no native softplus table
    in this compiler): exp(psA) underflows to exactly 0 for masked elements
    so ln(1+u) contributes 0 there; accum_out gives free row-sums.  Exp and
    Ln live in different ACT function tables, so ln passes are batched per
    chunk-group to avoid per-chunk table reloads.
  - The BCE "- x*z" term needs no extra pass structure: selected elements
    are always valid, so psA = x there, and one scalar_tensor_tensor
    psA * D (D the fp8 one-hot selector, read from SBUF) with accum_out
    yields sum_sel x exactly.
  - The small (T,B) losses (dir/mag/pid/charge/stop) are computed on-device
    from host-scattered dense planes (elementwise ops on GpSimd, transcend-
    entals phased into the main ACT batches); index bookkeeping (bincount,
    cumcount, scatter, argmax one-hots, denominators) is host-side numpy.
  - Per-core partial sums are returned and combined on the host in float64.
"""

import numpy as np

import concourse.bacc as bacc
import concourse.mybir as mybir
from concourse.tile import TileContext
from concourse.bass_utils import run_bass_kernel_spmd

F32 = mybir.dt.float32
BF16 = mybir.dt.bfloat16
F8 = mybir.dt.float8e4
NP_F8 = mybir.dt.np(F8)
NP_BF16 = mybir.dt.np(BF16)

T, B, N, NPFO = 32, 256, 500000, 4096
L_DIR, L_MAG, L_PID, L_CHG, L_ASN, L_STP = 1.0, 1.0, 1.0, 0.5, 1.0, 0.5

N_CORES = 8
H = N // N_CORES          # hits per core
J = 4                     # partition packing factor (J*T = 128)
HQ = H // J               # packed columns per core
P = J * T                 # 128 partitions
FCH = 2048                # chunk width (columns)
MMW = 512                 # one PSUM bank (512 f32 cols) per matmul
PEN = 96.0                # mask penalty; exp(x-96) underflows to 0
NGRP = 2                  # chunk groups; exp/ln batched per group (ACT tables)

_CHUNKS = []
_c0 = 0
while _c0 < HQ:
    _CHUNKS.append((_c0, min(FCH, HQ - _c0)))
    _c0 += FCH
NCH = len(_CHUNKS)
assert NCH <= 16
_GROUPS = [
    list(range(g * NCH // NGRP, (g + 1) * NCH // NGRP)) for g in range(NGRP)
]
_GW = max(sum(_CHUNKS[ci][1] for ci in grp) for grp in _GROUPS)

# small-loss planes, each (T*B,) flattened to (128, 64)
_PLANES = [
    "pm0", "pm1", "pm2", "gm0", "gm1", "gm2", "pp", "gp", "pch", "gch",
    "stopx", "stopz", "valid",
    "pid0", "pid1", "pid2", "pid3", "pid4",
    "poh0", "poh1", "poh2", "poh3", "poh4",
]
NPL = len(_PLANES)
SW = 64  # small-plane free width (T*B = 8192 = 128*64)

_nc_cache = None
last_result = None


def _gen():
    nc = bacc.Bacc(None, target_bir_lowering=False, debug=True)
    xh = nc.dram_tensor("xh", [P, HQ], BF16, kind="ExternalInput")
    xl = nc.dram_tensor("xl", [P, HQ], BF16, kind="ExternalInput")
    e8 = nc.dram_tensor("e8", [P, HQ], F8, kind="ExternalInput")
    d8 = nc.dram_tensor("d8", [P, HQ], F8, kind="ExternalInput")
    l8 = nc.dram_tensor("l8", [P, P], F8, kind="ExternalInput")
    ibf = nc.dram_tensor("ibf", [P, P], BF16, kind="ExternalInput")
    sm = nc.dram_tensor("sm", [P, NPL * SW], F32, kind="ExternalInput")
    partials = nc.dram_tensor("partials", [P, 40], F32, kind="ExternalOutput")

    AF = mybir.ActivationFunctionType
    OP = mybir.AluOpType

    with TileContext(nc) as tc:
        with (
            tc.tile_pool(name="cst", bufs=1) as cst,
            tc.tile_pool(name="io", bufs=3) as io,
            tc.tile_pool(name="wk", bufs=2) as wk,
            tc.tile_pool(name="ps", bufs=2, space="PSUM") as ps,
            tc.tile_pool(name="sml", bufs=1) as sml,
        ):
            lt = cst.tile([P, P], F8)
            ft = cst.tile([P, P], BF16)
            accA = cst.tile([P, 16], F32)
            accB = cst.tile([P, 16], F32)
            accS = cst.tile([P, 8], F32)
            ubuf = cst.tile([P, _GW], F32)
            nc.sync.dma_start(out=lt[:], in_=l8[:])
            nc.sync.dma_start(out=ft[:], in_=ibf[:])
            nc.vector.memset(accA[:], 0.0)
            nc.vector.memset(accB[:], 0.0)
            nc.vector.memset(accS[:], 0.0)

            # -------- small (T,B) losses: planes + elementwise on GpSimd ----
            smt = sml.tile([P, NPL * SW], F32)
            nc.sync.dma_start(out=smt[:], in_=sm[:])
            pl = {n: smt[:, i * SW : (i + 1) * SW] for i, n in enumerate(_PLANES)}

            _tmp_n = [0]

            def tmp():
                _tmp_n[0] += 1
                nm = f"tmp{_tmp_n[0]}"
                return sml.tile([P, SW], F32, name=nm, tag=nm)

            g = nc.vector  # gpsimd TT rejected by walrus codegen (Pool engine check)

            def sumsq3(c0_, c1_, c2_):
                ss = tmp()
                t1 = tmp()
                g.tensor_mul(ss[:], c0_, c0_)
                g.tensor_mul(t1[:], c1_, c1_)
                g.tensor_add(ss[:], ss[:], t1[:])
                g.tensor_mul(t1[:], c2_, c2_)
                g.tensor_add(ss[:], ss[:], t1[:])
                return ss

            # --- ACT-free elementwise prep
            ssp = sumsq3(pl["pm0"], pl["pm1"], pl["pm2"])
            ssg = sumsq3(pl["gm0"], pl["gm1"], pl["gm2"])
            dot = tmp()
            t2 = tmp()
            g.tensor_mul(dot[:], pl["pm0"], pl["gm0"])
            g.tensor_mul(t2[:], pl["pm1"], pl["gm1"])
            g.tensor_add(dot[:], dot[:], t2[:])
            g.tensor_mul(t2[:], pl["pm2"], pl["gm2"])
            g.tensor_add(dot[:], dot[:], t2[:])

            pid = [pl[f"pid{k}"] for k in range(5)]
            poh = [pl[f"poh{k}"] for k in range(5)]
            pm = tmp()
            g.tensor_max(pm[:], pid[0], pid[1])
            for k in range(2, 5):
                g.tensor_max(pm[:], pm[:], pid[k])
            pids = [tmp() for _ in range(5)]
            for k in range(5):
                g.tensor_sub(pids[k][:], pid[k], pm[:])
            xcls = tmp()
            tk = tmp()
            g.tensor_mul(xcls[:], pid[0], poh[0])
            for k in range(1, 5):
                g.tensor_mul(tk[:], pid[k], poh[k])
                g.tensor_add(xcls[:], xcls[:], tk[:])
            xz = tmp()
            g.tensor_mul(xz[:], pl["stopx"], pl["stopz"])

            # mag / charge need no ACT at all
            def masked_sq(a, b, col):
                dd = tmp()
                g.tensor_sub(dd[:], a, b)
                g.tensor_mul(dd[:], dd[:], dd[:])
                o = tmp()
                nc.vector.scalar_tensor_tensor(
                    out=o[:], in0=dd[:], scalar=1.0, in1=pl["valid"],
                    op0=OP.mult, op1=OP.mult, accum_out=accS[:, col : col + 1],
                )

            masked_sq(pl["pp"], pl["gp"], 1)       # magnitude
            masked_sq(pl["pch"], pl["gch"], 2)     # charge

            # deferred small-ACT pieces, emitted inside the main phases
            pide = [tmp() for _ in range(5)]       # exp(pid_k - m)
            stope = tmp()                          # exp(stopx)
            lnp = tmp()                            # ln(ssp)
            lng = tmp()                            # ln(ssg)
            se = tmp()                             # sum exp
            lse = tmp()                            # ln(se)
            spv = tmp()                            # ln(1+exp(stopx))
            srp = tmp()                            # sqrt(ssp)
            srg = tmp()                            # sqrt(ssg)

            def smalls_exp_g0():
                for k in range(5):
                    nc.scalar.activation(out=pide[k][:], in_=pids[k][:],
                                         func=AF.Exp)
                nc.scalar.activation(out=stope[:], in_=pl["stopx"],
                                     func=AF.Exp)

            def smalls_ln_g0():
                nc.scalar.activation(out=lnp[:], in_=ssp[:], func=AF.Ln)
                nc.scalar.activation(out=lng[:], in_=ssg[:], func=AF.Ln)
                g.tensor_add(se[:], pide[0][:], pide[1][:])
                for k in range(2, 5):
                    g.tensor_add(se[:], se[:], pide[k][:])
                nc.scalar.activation(out=lse[:], in_=se[:], func=AF.Ln)
                nc.scalar.activation(out=spv[:], in_=stope[:], func=AF.Ln,
                                     bias=1.0)

            def smalls_exp_g1():
                nc.scalar.activation(out=srp[:], in_=lnp[:], func=AF.Exp,
                                     scale=0.5)
                nc.scalar.activation(out=srg[:], in_=lng[:], func=AF.Exp,
                                     scale=0.5)

            def smalls_finish():
                # dir: cos = dot / (max(srp,eps) * max(srg,eps))
                nc.vector.tensor_scalar(out=srp[:], in0=srp[:], scalar1=1e-8,
                                        scalar2=None, op0=OP.max)
                nc.vector.tensor_scalar(out=srg[:], in0=srg[:], scalar1=1e-8,
                                        scalar2=None, op0=OP.max)
                nc.vector.reciprocal(out=srp[:], in_=srp[:])
                nc.vector.reciprocal(out=srg[:], in_=srg[:])
                g.tensor_mul(dot[:], dot[:], srp[:])
                g.tensor_mul(dot[:], dot[:], srg[:])
                cv = tmp()
                g.tensor_mul(cv[:], dot[:], pl["valid"])
                o1 = tmp()
                nc.vector.scalar_tensor_tensor(
                    out=o1[:], in0=cv[:], scalar=-1.0, in1=pl["valid"],
                    op0=OP.mult, op1=OP.add, accum_out=accS[:, 0:1],
                )
                # pid: (lse + m - xcls) * valid
                n1 = tmp()
                g.tensor_add(n1[:], lse[:], pm[:])
                u1 = tmp()
                nc.vector.scalar_tensor_tensor(
                    out=u1[:], in0=xcls[:], scalar=-1.0, in1=n1[:],
                    op0=OP.mult, op1=OP.add,
                )
                o2 = tmp()
                nc.vector.scalar_tensor_tensor(
                    out=o2[:], in0=u1[:], scalar=1.0, in1=pl["valid"],
                    op0=OP.mult, op1=OP.mult, accum_out=accS[:, 3:4],
                )
                # stop: softplus(x) - x*z
                o3 = tmp()
                nc.vector.scalar_tensor_tensor(
                    out=o3[:], in0=xz[:], scalar=-1.0, in1=spv[:],
                    op0=OP.mult, op1=OP.add, accum_out=accS[:, 4:5],
                )

            # ---------------- main loop: assignment loss ----------------
            for gi, grp in enumerate(_GROUPS):
                goff = 0
                for ci in grp:
                    c0, w = _CHUNKS[ci]
                    xht = io.tile([P, FCH], BF16, tag="xht")
                    xlt = io.tile([P, FCH], BF16, tag="xlt")
                    et = io.tile([P, FCH], F8, tag="et")
                    dt = io.tile([P, FCH], F8, tag="dt")
                    nc.sync.dma_start(out=xht[:, :w], in_=xh[:, c0 : c0 + w])
                    nc.sync.dma_start(out=xlt[:, :w], in_=xl[:, c0 : c0 + w])
                    nc.sync.dma_start(out=et[:, :w], in_=e8[:, c0 : c0 + w])
                    nc.sync.dma_start(out=dt[:, :w], in_=d8[:, c0 : c0 + w])

                    psA = ps.tile([P, FCH], F32, tag="psA")
                    h0 = 0
                    while h0 < w:
                        hw = min(MMW, w - h0)
                        sl = slice(h0, h0 + hw)
                        nc.tensor.matmul(
                            psA[:, sl], lt[:], et[:, sl], start=True,
                            stop=False,
                        )
                        nc.tensor.matmul(
                            psA[:, sl], ft[:], xht[:, sl], start=False,
                            stop=False,
                        )
                        nc.tensor.matmul(
                            psA[:, sl], ft[:], xlt[:, sl], start=False,
                            stop=True,
                        )
                        h0 += hw

                    nc.scalar.activation(
                        out=ubuf[:, goff : goff + w], in_=psA[:, :w],
                        func=AF.Exp,
                    )
                    rt = wk.tile([P, FCH], F32, tag="rt")
                    nc.vector.scalar_tensor_tensor(
                        out=rt[:, :w],
                        in0=psA[:, :w],
                        scalar=1.0,
                        in1=dt[:, :w],
                        op0=OP.mult,
                        op1=OP.mult,
                        accum_out=accB[:, ci : ci + 1],
                    )
                    goff += w
                if gi == 0:
                    smalls_exp_g0()
                elif gi == 1:
                    smalls_exp_g1()
                goff = 0
                for ci in grp:
                    c0, w = _CHUNKS[ci]
                    st = wk.tile([P, FCH], F32, tag="st")
                    nc.scalar.activation(
                        out=st[:, :w],
                        in_=ubuf[:, goff : goff + w],
                        func=AF.Ln,
                        bias=1.0,
                        accum_out=accA[:, ci : ci + 1],
                    )
                    goff += w
                if gi == 0:
                    smalls_ln_g0()

            smalls_finish()

            nc.sync.dma_start(out=partials[:, 0:16], in_=accA[:])
            nc.sync.dma_start(out=partials[:, 16:32], in_=accB[:])
            nc.sync.dma_start(out=partials[:, 32:40], in_=accS[:])
    nc.finalize()
    return nc


def _get_nc():
    global _nc_cache
    if _nc_cache is None:
        _nc_cache = _gen()
    return _nc_cache


def _cumcount(gb):
    n = gb.shape[0]
    order = np.argsort(gb, kind="stable")
    sb = gb[order]
    first = np.searchsorted(sb, sb, side="left")
    cum = np.arange(n) - first
    out = np.zeros(n, dtype=np.int64)
    out[order] = cum
    return out


def kernel(**inputs):
    pfo_momentum = np.asarray(inputs["pfo_momentum"], np.float32)
    pfo_p_mod = np.asarray(inputs["pfo_p_mod"], np.float32)
    pfo_pid = np.asarray(inputs["pfo_pid"], np.float32)
    pfo_charge = np.asarray(inputs["pfo_charge"], np.float32)
    al = np.asarray(inputs["assignments_logits"], np.float32).reshape(T, N)
    stop_logits = np.asarray(inputs["stop_logits"], np.float32)
    gt_momentum = np.asarray(inputs["gt_momentum"], np.float32)
    gt_p_mod = np.asarray(inputs["gt_p_mod"], np.float32)
    gt_pid = np.asarray(inputs["gt_pid"], np.float32)
    gt_charge = np.asarray(inputs["gt_charge"], np.float32)
    gt_batch = np.asarray(inputs["gt_batch"]).astype(np.int64)
    hit_to_pfo = np.asarray(inputs["hit_to_pfo"]).astype(np.int64)
    hit_batch = np.asarray(inputs["hit_batch"]).astype(np.int64)

    # ---- host index bookkeeping ----
    ppe = np.bincount(gt_batch, minlength=B)[:B]                  # (B,)
    cmin = np.minimum(ppe[hit_batch], T)                          # (N,)
    w = hit_to_pfo < cmin                                         # (N,) bool
    assign_den = max(float(cmin.sum()), 1.0)

    step_idx = _cumcount(gt_batch)
    keep = step_idx < T
    si, gb = step_idx[keep], gt_batch[keep]

    def scat(vals):
        out = np.zeros((T, B) + vals.shape[1:], np.float32)
        out[si, gb] = vals[keep]
        return out

    gt_mom_tb = scat(gt_momentum)
    gt_pmod_tb = scat(gt_p_mod)
    gt_pid_tb = scat(gt_pid)
    gt_chg_tb = scat(gt_charge)

    steps = np.arange(T)[:, None]
    valid = (steps < ppe[None, :]).astype(np.float32)             # (T,B)
    vcnt = max(float(valid.sum()), 1.0)
    gt_stop = (steps >= ppe[None, :]).astype(np.float32)
    gt_cls = np.argmax(gt_pid_tb, axis=-1)                        # (T,B)
    poh = np.zeros((T, B, 5), np.float32)
    np.put_along_axis(poh, gt_cls[..., None], 1.0, axis=-1)

    # ---- per-core device inputs ----
    def pack_plane(a):
        return np.ascontiguousarray(a.reshape(P, SW))

    planes = {
        "pm0": pfo_momentum[..., 0], "pm1": pfo_momentum[..., 1],
        "pm2": pfo_momentum[..., 2],
        "gm0": gt_mom_tb[..., 0], "gm1": gt_mom_tb[..., 1],
        "gm2": gt_mom_tb[..., 2],
        "pp": pfo_p_mod[..., 0], "gp": gt_pmod_tb[..., 0],
        "pch": pfo_charge[..., 0], "gch": gt_chg_tb[..., 0],
        "stopx": stop_logits[..., 0], "stopz": gt_stop,
        "valid": valid,
        **{f"pid{k}": pfo_pid[..., k] for k in range(5)},
        **{f"poh{k}": poh[..., k] for k in range(5)},
    }
    sm = np.concatenate([pack_plane(planes[n]) for n in _PLANES], axis=1)

    l8 = np.zeros((P, P), np.float32)
    for j in range(J):
        blk = -PEN * np.tril(np.ones((T, T), np.float32)).T  # [k,t] = -96*(t>=k)
        l8[j * T : (j + 1) * T, j * T : (j + 1) * T] = blk
    l8 = l8.astype(NP_F8)
    ibf = np.eye(P, dtype=np.float32).astype(NP_BF16)

    # one-hot E (mask count) and D (selection) per core, fp8
    cj = cmin.reshape(N_CORES, J, HQ)
    pj = hit_to_pfo.reshape(N_CORES, J, HQ)
    wj = w.reshape(N_CORES, J, HQ)
    in_maps = []
    for c in range(N_CORES):
        E = np.zeros((P, HQ), NP_F8)
        D = np.zeros((P, HQ), NP_F8)
        for j in range(J):
            cc = cj[c, j]
            me = cc < T
            fs = np.nonzero(me)[0]
            E[j * T + cc[fs], fs] = 1.0
            fs = np.nonzero(wj[c, j])[0]
            D[j * T + pj[c, j][fs], fs] = 1.0
        xs = al[:, c * H : (c + 1) * H].reshape(T, J, HQ)
        xp = np.ascontiguousarray(xs.transpose(1, 0, 2).reshape(P, HQ))
        xhp = xp.astype(NP_BF16)
        xlp = (xp - xhp.astype(np.float32)).astype(NP_BF16)
        in_maps.append(
            {"xh": xhp, "xl": xlp, "e8": E, "d8": D, "l8": l8, "ibf": ibf,
             "sm": sm}
        )

    nc = _get_nc()
    res = run_bass_kernel_spmd(nc, in_maps, core_ids=list(range(N_CORES)))
    global last_result
    last_result = res

    # ---- host combine (float64) ----
    A_sum = 0.0
    B_sum = 0.0
    for c in range(N_CORES):
        pr = res.results[c]["partials"].astype(np.float64)
        A_sum += pr[:, 0:16].sum()
        B_sum += pr[:, 16:32].sum()
    loss_assign = (A_sum - B_sum) / assign_den

    pr0 = res.results[0]["partials"].astype(np.float64)
    loss_dir = pr0[:, 32].sum() / vcnt
    loss_mag = pr0[:, 33].sum() / vcnt
    loss_chg = pr0[:, 34].sum() / vcnt
    loss_pid = pr0[:, 35].sum() / vcnt
    loss_stop = pr0[:, 36].sum() / (T * B)

    total = (L_DIR * loss_dir + L_MAG * loss_mag + L_PID * loss_pid
             + L_CHG * loss_chg + L_ASN * loss_assign + L_STP * loss_stop)
    f = np.float32
    return (f(total), f(loss_dir), f(loss_mag), f(loss_pid), f(loss_chg),
            f(loss_assign), f(loss_stop))


# revision 10
# speedup vs baseline: 1.7643x; 1.2349x over previous
"""Trainium2 Bass kernel for nn_GATrAutoRegressorLoss.

Strategy (data-parallel over the hit axis N, 8 cores):
  - The dominant cost is the assignment BCE over (T=32, N=500000) logits.
    Each core gets H = N/8 = 62500 hits, packed as a (128, 15625) layout:
    partition p = j*32 + t, column f, hit = j*15625 + f.
  - The validity mask is folded into the logits on the PE: host-built fp8
    one-hot columns E (encoding c(hit) = #valid steps) hit a constant
    block-triangular L with value -96, accumulating -96*(t >= c) into PSUM;
    x rides in via two bf16 identity matmuls (hi + lo split preserves f32
    precision).  psA = x - 96*notM.
  - softplus = ln(1 + exp(.)) as two ACT passes (no native softplus table
    in this compiler): exp(psA) underflows to exactly 0 for masked elements
    so ln(1+u) contributes 0 there; accum_out gives free row-sums.  Exp and
    Ln live in different ACT function tables, so ln passes are batched per
    chunk-group to avoid per-chunk table reloads.
  - The BCE "- x*z" term needs no extra pass structure: selected elements
    are always valid, so psA = x there, and one scalar_tensor_tensor
    psA * D (D the fp8 one-hot selector, read from SBUF) with accum_out
    yields sum_sel x exactly.
  - The small (T,B) losses (dir/mag/pid/charge/stop) are computed on-device
    from host-scattered dense planes (elementwise ops on GpSimd, transcend-
    entals phased into the main ACT batches); index bookkeeping (bincount,
    cumcount, scatter, argmax one-hots, denominators) is host-side numpy.
  - Per-core partial sums are returned and combined on the host in float64.
"""

import numpy as np

import concourse.bacc as bacc
import concourse.mybir as mybir
from concourse.tile import TileContext
from concourse.bass_utils import run_bass_kernel_spmd

F32 = mybir.dt.float32
BF16 = mybir.dt.bfloat16
F8 = mybir.dt.float8e4
NP_F8 = mybir.dt.np(F8)
NP_BF16 = mybir.dt.np(BF16)

T, B, N, NPFO = 32, 256, 500000, 4096
L_DIR, L_MAG, L_PID, L_CHG, L_ASN, L_STP = 1.0, 1.0, 1.0, 0.5, 1.0, 0.5

N_CORES = 8
H = N // N_CORES          # hits per core
J = 4                     # partition packing factor (J*T = 128)
HQ = H // J               # packed columns per core
P = J * T                 # 128 partitions
FCH = 2048                # chunk width (columns)
MMW = 512                 # one PSUM bank (512 f32 cols) per matmul
PEN = 96.0                # mask penalty; exp(x-96) underflows to 0
NGRP = 2                  # chunk groups; exp/ln batched per group (ACT tables)

_CHUNKS = []
_c0 = 0
while _c0 < HQ:
    _CHUNKS.append((_c0, min(FCH, HQ - _c0)))
    _c0 += FCH
NCH = len(_CHUNKS)
assert NCH <= 16
_GROUPS = [
    list(range(g * NCH // NGRP, (g + 1) * NCH // NGRP)) for g in range(NGRP)
]
_GW = max(sum(_CHUNKS[ci][1] for ci in grp) for grp in _GROUPS)

# small-loss planes, each (T*B,) flattened to (128, 64)
_PLANES = [
    "pm0", "pm1", "pm2", "gm0", "gm1", "gm2", "pp", "gp", "pch", "gch",
    "stopx", "stopz", "valid",
    "pid0", "pid1", "pid2", "pid3", "pid4",
    "poh0", "poh1", "poh2", "poh3", "poh4",
]
NPL = len(_PLANES)
SW = 64  # small-plane free width (T*B = 8192 = 128*64)

_nc_cache = None
last_result = None


class _Bacc(bacc.Bacc):
    """Bacc whose ACT-table chooser binds Exp/Ln to the one json table that
    contains both (natural_log_exp_and_others), so the Scalar engine never
    reloads function tables between exp and ln passes.  Table ids passed to
    the rust pass keep their act_info.json positions; only the advertised
    contents are narrowed, so codegen still loads the real (correct) table."""

    def insert_act_table_loads(self):
        from concourse.hw_specs import get_activation_tables

        has_activation = any(
            isinstance(i, mybir.InstActivation)
            for b in self.main_func.blocks
            for i in b.instructions
        )
        if not has_activation:
            return
        AF = mybir.ActivationFunctionType
        tables = []
        for name, fns in get_activation_tables(self.m.arch).items():
            if name != "natural_log_exp_and_others":
                fns = set(fns) - {AF.Exp, AF.Ln}
            tables.append((name, set(fns)))
        import bass_rust as _bass_rust

        _bass_rust.insert_act_table_loads(self, tables)


def _gen():
    nc = _Bacc(None, target_bir_lowering=False, debug=True)
    xh = nc.dram_tensor("xh", [P, HQ], BF16, kind="ExternalInput")
    e8 = nc.dram_tensor("e8", [P, HQ], F8, kind="ExternalInput")
    d8 = nc.dram_tensor("d8", [P, HQ], F8, kind="ExternalInput")
    l8 = nc.dram_tensor("l8", [P, P], F8, kind="ExternalInput")
    ibf = nc.dram_tensor("ibf", [P, P], BF16, kind="ExternalInput")
    sm = nc.dram_tensor("sm", [P, NPL * SW], F32, kind="ExternalInput")
    partials = nc.dram_tensor("partials", [P, 40], F32, kind="ExternalOutput")

    AF = mybir.ActivationFunctionType
    OP = mybir.AluOpType

    with TileContext(nc) as tc:
        with (
            tc.tile_pool(name="cst", bufs=1) as cst,
            tc.tile_pool(name="io", bufs=3) as io,
            tc.tile_pool(name="wk", bufs=2) as wk,
            tc.tile_pool(name="ps", bufs=2, space="PSUM") as ps,
            tc.tile_pool(name="sml", bufs=1) as sml,
        ):
            lt = cst.tile([P, P], F8)
            ft = cst.tile([P, P], BF16)
            accA = cst.tile([P, 16], F32)
            accB = cst.tile([P, 16], F32)
            accS = cst.tile([P, 8], F32)
            ebuf = cst.tile([P, HQ], F8)
            dbuf = cst.tile([P, HQ], F8)
            nc.sync.dma_start(out=lt[:], in_=l8[:])
            nc.sync.dma_start(out=ft[:], in_=ibf[:])
            nc.sync.dma_start(out=ebuf[:], in_=e8[:])
            nc.sync.dma_start(out=dbuf[:], in_=d8[:])
            nc.vector.memset(accA[:], 0.0)
            nc.vector.memset(accB[:], 0.0)
            nc.vector.memset(accS[:], 0.0)

            # -------- small (T,B) losses: planes + elementwise on GpSimd ----
            smt = sml.tile([P, NPL * SW], F32)
            nc.sync.dma_start(out=smt[:], in_=sm[:])
            pl = {n: smt[:, i * SW : (i + 1) * SW] for i, n in enumerate(_PLANES)}

            _tmp_n = [0]

            def tmp():
                _tmp_n[0] += 1
                nm = f"tmp{_tmp_n[0]}"
                return sml.tile([P, SW], F32, name=nm, tag=nm)

            g = nc.vector  # gpsimd TT rejected by walrus codegen (Pool engine check)

            def sumsq3(c0_, c1_, c2_):
                ss = tmp()
                t1 = tmp()
                g.tensor_mul(ss[:], c0_, c0_)
                g.tensor_mul(t1[:], c1_, c1_)
                g.tensor_add(ss[:], ss[:], t1[:])
                g.tensor_mul(t1[:], c2_, c2_)
                g.tensor_add(ss[:], ss[:], t1[:])
                return ss

            # --- ACT-free elementwise prep
            ssp = sumsq3(pl["pm0"], pl["pm1"], pl["pm2"])
            ssg = sumsq3(pl["gm0"], pl["gm1"], pl["gm2"])
            dot = tmp()
            t2 = tmp()
            g.tensor_mul(dot[:], pl["pm0"], pl["gm0"])
            g.tensor_mul(t2[:], pl["pm1"], pl["gm1"])
            g.tensor_add(dot[:], dot[:], t2[:])
            g.tensor_mul(t2[:], pl["pm2"], pl["gm2"])
            g.tensor_add(dot[:], dot[:], t2[:])

            pid = [pl[f"pid{k}"] for k in range(5)]
            poh = [pl[f"poh{k}"] for k in range(5)]
            pm = tmp()
            g.tensor_max(pm[:], pid[0], pid[1])
            for k in range(2, 5):
                g.tensor_max(pm[:], pm[:], pid[k])
            pids = [tmp() for _ in range(5)]
            for k in range(5):
                g.tensor_sub(pids[k][:], pid[k], pm[:])
            xcls = tmp()
            tk = tmp()
            g.tensor_mul(xcls[:], pid[0], poh[0])
            for k in range(1, 5):
                g.tensor_mul(tk[:], pid[k], poh[k])
                g.tensor_add(xcls[:], xcls[:], tk[:])
            xz = tmp()
            g.tensor_mul(xz[:], pl["stopx"], pl["stopz"])

            # mag / charge need no ACT at all
            def masked_sq(a, b, col):
                dd = tmp()
                g.tensor_sub(dd[:], a, b)
                g.tensor_mul(dd[:], dd[:], dd[:])
                o = tmp()
                nc.vector.scalar_tensor_tensor(
                    out=o[:], in0=dd[:], scalar=1.0, in1=pl["valid"],
                    op0=OP.mult, op1=OP.mult, accum_out=accS[:, col : col + 1],
                )

            masked_sq(pl["pp"], pl["gp"], 1)       # magnitude
            masked_sq(pl["pch"], pl["gch"], 2)     # charge

            # deferred small-ACT pieces, emitted inside the main phases
            pide = [tmp() for _ in range(5)]       # exp(pid_k - m)
            stope = tmp()                          # exp(stopx)
            lnp = tmp()                            # ln(ssp)
            lng = tmp()                            # ln(ssg)
            se = tmp()                             # sum exp
            lse = tmp()                            # ln(se)
            spv = tmp()                            # ln(1+exp(stopx))
            srp = tmp()                            # sqrt(ssp)
            srg = tmp()                            # sqrt(ssg)

            def smalls_exp_g0():
                for k in range(5):
                    nc.scalar.activation(out=pide[k][:], in_=pids[k][:],
                                         func=AF.Exp)
                nc.scalar.activation(out=stope[:], in_=pl["stopx"],
                                     func=AF.Exp)

            def smalls_ln_g0():
                nc.scalar.activation(out=lnp[:], in_=ssp[:], func=AF.Ln)
                nc.scalar.activation(out=lng[:], in_=ssg[:], func=AF.Ln)
                g.tensor_add(se[:], pide[0][:], pide[1][:])
                for k in range(2, 5):
                    g.tensor_add(se[:], se[:], pide[k][:])
                nc.scalar.activation(out=lse[:], in_=se[:], func=AF.Ln)
                nc.scalar.activation(out=spv[:], in_=stope[:], func=AF.Ln,
                                     bias=1.0)

            def smalls_exp_g1():
                nc.scalar.activation(out=srp[:], in_=lnp[:], func=AF.Exp,
                                     scale=0.5)
                nc.scalar.activation(out=srg[:], in_=lng[:], func=AF.Exp,
                                     scale=0.5)

            def smalls_finish():
                # dir: cos = dot / (max(srp,eps) * max(srg,eps))
                nc.vector.tensor_scalar(out=srp[:], in0=srp[:], scalar1=1e-8,
                                        scalar2=None, op0=OP.max)
                nc.vector.tensor_scalar(out=srg[:], in0=srg[:], scalar1=1e-8,
                                        scalar2=None, op0=OP.max)
                nc.vector.reciprocal(out=srp[:], in_=srp[:])
                nc.vector.reciprocal(out=srg[:], in_=srg[:])
                g.tensor_mul(dot[:], dot[:], srp[:])
                g.tensor_mul(dot[:], dot[:], srg[:])
                cv = tmp()
                g.tensor_mul(cv[:], dot[:], pl["valid"])
                o1 = tmp()
                nc.vector.scalar_tensor_tensor(
                    out=o1[:], in0=cv[:], scalar=-1.0, in1=pl["valid"],
                    op0=OP.mult, op1=OP.add, accum_out=accS[:, 0:1],
                )
                # pid: (lse + m - xcls) * valid
                n1 = tmp()
                g.tensor_add(n1[:], lse[:], pm[:])
                u1 = tmp()
                nc.vector.scalar_tensor_tensor(
                    out=u1[:], in0=xcls[:], scalar=-1.0, in1=n1[:],
                    op0=OP.mult, op1=OP.add,
                )
                o2 = tmp()
                nc.vector.scalar_tensor_tensor(
                    out=o2[:], in0=u1[:], scalar=1.0, in1=pl["valid"],
                    op0=OP.mult, op1=OP.mult, accum_out=accS[:, 3:4],
                )
                # stop: softplus(x) - x*z
                o3 = tmp()
                nc.vector.scalar_tensor_tensor(
                    out=o3[:], in0=xz[:], scalar=-1.0, in1=spv[:],
                    op0=OP.mult, op1=OP.add, accum_out=accS[:, 4:5],
                )

            # ---------------- main loop: assignment loss ----------------
            for ci, (c0, w) in enumerate(_CHUNKS):
                xht = io.tile([P, FCH], BF16, tag="xht")
                nc.sync.dma_start(out=xht[:, :w], in_=xh[:, c0 : c0 + w])

                psA = ps.tile([P, FCH], F32, tag="psA")
                h0 = 0
                while h0 < w:
                    hw = min(MMW, w - h0)
                    sl = slice(h0, h0 + hw)
                    esl = slice(c0 + h0, c0 + h0 + hw)
                    nc.tensor.matmul(
                        psA[:, sl], lt[:], ebuf[:, esl], start=True, stop=False
                    )
                    nc.tensor.matmul(
                        psA[:, sl], ft[:], xht[:, sl], start=False, stop=True
                    )
                    h0 += hw

                ut = wk.tile([P, FCH], F32, tag="ut")
                st = wk.tile([P, FCH], F32, tag="st")
                nc.scalar.activation(out=ut[:, :w], in_=psA[:, :w], func=AF.Exp)
                nc.scalar.activation(
                    out=st[:, :w],
                    in_=ut[:, :w],
                    func=AF.Ln,
                    bias=1.0,
                    accum_out=accA[:, ci : ci + 1],
                )
                rt = wk.tile([P, FCH], F32, tag="rt")
                nc.vector.scalar_tensor_tensor(
                    out=rt[:, :w],
                    in0=xht[:, :w],
                    scalar=1.0,
                    in1=dbuf[:, c0 : c0 + w],
                    op0=OP.mult,
                    op1=OP.mult,
                    accum_out=accB[:, ci : ci + 1],
                )
            smalls_exp_g0()
            smalls_ln_g0()
            smalls_exp_g1()
            smalls_finish()

            nc.sync.dma_start(out=partials[:, 0:16], in_=accA[:])
            nc.sync.dma_start(out=partials[:, 16:32], in_=accB[:])
            nc.sync.dma_start(out=partials[:, 32:40], in_=accS[:])
    nc.finalize()
    return nc


def _get_nc():
    global _nc_cache
    if _nc_cache is None:
        _nc_cache = _gen()
    return _nc_cache


def _cumcount(gb):
    n = gb.shape[0]
    order = np.argsort(gb, kind="stable")
    sb = gb[order]
    first = np.searchsorted(sb, sb, side="left")
    cum = np.arange(n) - first
    out = np.zeros(n, dtype=np.int64)
    out[order] = cum
    return out


def kernel(**inputs):
    pfo_momentum = np.asarray(inputs["pfo_momentum"], np.float32)
    pfo_p_mod = np.asarray(inputs["pfo_p_mod"], np.float32)
    pfo_pid = np.asarray(inputs["pfo_pid"], np.float32)
    pfo_charge = np.asarray(inputs["pfo_charge"], np.float32)
    al = np.asarray(inputs["assignments_logits"], np.float32).reshape(T, N)
    stop_logits = np.asarray(inputs["stop_logits"], np.float32)
    gt_momentum = np.asarray(inputs["gt_momentum"], np.float32)
    gt_p_mod = np.asarray(inputs["gt_p_mod"], np.float32)
    gt_pid = np.asarray(inputs["gt_pid"], np.float32)
    gt_charge = np.asarray(inputs["gt_charge"], np.float32)
    gt_batch = np.asarray(inputs["gt_batch"]).astype(np.int64)
    hit_to_pfo = np.asarray(inputs["hit_to_pfo"]).astype(np.int64)
    hit_batch = np.asarray(inputs["hit_batch"]).astype(np.int64)

    # ---- host index bookkeeping ----
    ppe = np.bincount(gt_batch, minlength=B)[:B]                  # (B,)
    cmin = np.minimum(ppe[hit_batch], T)                          # (N,)
    w = hit_to_pfo < cmin                                         # (N,) bool
    assign_den = max(float(cmin.sum()), 1.0)

    step_idx = _cumcount(gt_batch)
    keep = step_idx < T
    si, gb = step_idx[keep], gt_batch[keep]

    def scat(vals):
        out = np.zeros((T, B) + vals.shape[1:], np.float32)
        out[si, gb] = vals[keep]
        return out

    gt_mom_tb = scat(gt_momentum)
    gt_pmod_tb = scat(gt_p_mod)
    gt_pid_tb = scat(gt_pid)
    gt_chg_tb = scat(gt_charge)

    steps = np.arange(T)[:, None]
    valid = (steps < ppe[None, :]).astype(np.float32)             # (T,B)
    vcnt = max(float(valid.sum()), 1.0)
    gt_stop = (steps >= ppe[None, :]).astype(np.float32)
    gt_cls = np.argmax(gt_pid_tb, axis=-1)                        # (T,B)
    poh = np.zeros((T, B, 5), np.float32)
    np.put_along_axis(poh, gt_cls[..., None], 1.0, axis=-1)

    # ---- per-core device inputs ----
    def pack_plane(a):
        return np.ascontiguousarray(a.reshape(P, SW))

    planes = {
        "pm0": pfo_momentum[..., 0], "pm1": pfo_momentum[..., 1],
        "pm2": pfo_momentum[..., 2],
        "gm0": gt_mom_tb[..., 0], "gm1": gt_mom_tb[..., 1],
        "gm2": gt_mom_tb[..., 2],
        "pp": pfo_p_mod[..., 0], "gp": gt_pmod_tb[..., 0],
        "pch": pfo_charge[..., 0], "gch": gt_chg_tb[..., 0],
        "stopx": stop_logits[..., 0], "stopz": gt_stop,
        "valid": valid,
        **{f"pid{k}": pfo_pid[..., k] for k in range(5)},
        **{f"poh{k}": poh[..., k] for k in range(5)},
    }
    sm = np.concatenate([pack_plane(planes[n]) for n in _PLANES], axis=1)

    l8 = np.zeros((P, P), np.float32)
    for j in range(J):
        blk = -PEN * np.tril(np.ones((T, T), np.float32)).T  # [k,t] = -96*(t>=k)
        l8[j * T : (j + 1) * T, j * T : (j + 1) * T] = blk
    l8 = l8.astype(NP_F8)
    ibf = np.eye(P, dtype=np.float32).astype(NP_BF16)

    # one-hot E (mask count) and D (selection) per core, fp8
    cj = cmin.reshape(N_CORES, J, HQ)
    pj = hit_to_pfo.reshape(N_CORES, J, HQ)
    wj = w.reshape(N_CORES, J, HQ)
    in_maps = []
    for c in range(N_CORES):
        E = np.zeros((P, HQ), NP_F8)
        D = np.zeros((P, HQ), NP_F8)
        for j in range(J):
            cc = cj[c, j]
            me = cc < T
            fs = np.nonzero(me)[0]
            E[j * T + cc[fs], fs] = 1.0
            fs = np.nonzero(wj[c, j])[0]
            D[j * T + pj[c, j][fs], fs] = 1.0
        xs = al[:, c * H : (c + 1) * H].reshape(T, J, HQ)
        xp = np.ascontiguousarray(xs.transpose(1, 0, 2).reshape(P, HQ))
        xhp = xp.astype(NP_BF16)
        in_maps.append(
            {"xh": xhp, "e8": E, "d8": D, "l8": l8, "ibf": ibf, "sm": sm}
        )

    nc = _get_nc()
    res = run_bass_kernel_spmd(nc, in_maps, core_ids=list(range(N_CORES)))
    global last_result
    last_result = res

    # ---- host combine (float64) ----
    A_sum = 0.0
    B_sum = 0.0
    for c in range(N_CORES):
        pr = res.results[c]["partials"].astype(np.float64)
        A_sum += pr[:, 0:16].sum()
        B_sum += pr[:, 16:32].sum()
    loss_assign = (A_sum - B_sum) / assign_den

    pr0 = res.results[0]["partials"].astype(np.float64)
    loss_dir = pr0[:, 32].sum() / vcnt
    loss_mag = pr0[:, 33].sum() / vcnt
    loss_chg = pr0[:, 34].sum() / vcnt
    loss_pid = pr0[:, 35].sum() / vcnt
    loss_stop = pr0[:, 36].sum() / (T * B)

    total = (L_DIR * loss_dir + L_MAG * loss_mag + L_PID * loss_pid
             + L_CHG * loss_chg + L_ASN * loss_assign + L_STP * loss_stop)
    f = np.float32
    return (f(total), f(loss_dir), f(loss_mag), f(loss_pid), f(loss_chg),
            f(loss_assign), f(loss_stop))
